# revision 35
# baseline (speedup 1.0000x reference)
"""Trainium2 Bass kernel for nn_MultiHeadedLinrec (linear attention), fp8 ed.

Math (per batch element, reference semantics):
    q = elu(x_q @ Wq.T + bq)    [S, E] viewed as [S, H, d]
    k = elu(x_k @ Wk.T + bk)
    v = x_v @ Wv.T + bv
    k <- k / (||k||_seq * sqrt(S))     (per (h, d) column norm over S)
    q <- q / (||q||_d   * sqrt(d))     (per (s, h) row norm over d)
    scores_h = k_h^T @ v_h             [d, d]
    out = concat_h(q_h @ scores_h) @ Wo.T + bo

Kernel strategy (one NeuronCore per batch element, 8 cores data-parallel):
  The four big S*E*E GEMMs (q/k/v projections and the fused output
  projection) run in fp8e4 with MatmulPerfMode.DoubleRow, which contracts
  two 128-row chunks per pass at 0.5 cyc/row.  Each GEMM uses a 3-pass
  hi/lo split (y ~= x_hi@w_hi + x_lo@w_hi + x_hi@w_lo, dropping the lo*lo
  term), giving bf16-level accuracy at 0.75x the bf16 matmul cost.

  The PE sequencer, not the PE array, is the binding resource at fp8
  speeds (~134 ns of SEQ per ldweights+matmul pair vs ~107 ns of array
  time), so matmuls are emitted stationary-major: all matmuls sharing a
  stationary operand are adjacent (both output halves x {w_hi, w_lo}),
  and _dedup_ldweights() then deletes the redundant auto-inserted
  InstLdweights, verified bit-exact on hardware.

  Scaling: weights are pre-scaled by 32 on the host so their std is ~1
  (fp8e4 normals bottom out at 2^-6); the 1/32 descale folds into the
  ELU/copy activations.  qn is scaled by 64 and W2 by 256 (both folded
  into the norm-reciprocal activations for free); the final output copy
  descales by 1/16384.

  Host pre-tiles activations as [p, blk, c, s] so every x-block DMA is a
  single contiguous 4KB-per-partition transfer, and pre-interleaves the
  fp8 weights in DoubleRow pair layout [g*128+p, (i n)].

  Everything else keeps the proven bf16 structure: per-head [v | k] bf16
  packing with PE-accumulated scoresT + k-gram (phase A), the fused
  W2[i,o] = (scores @ Wo.T)[i,o] * invk[i] weight (phase B, bf16 matmuls,
  fp8 hi/lo output), and the software-pipelined q pass (phase C).

This walrus build only supports ONE sync wait per instruction; Tile emits
multi-wait instructions, so we legalize the BIR JSON by hoisting extra waits
onto inserted NoOps (see _legalize_sync_json).
"""

import json

import numpy as np

import concourse.bass as bass
import concourse.mybir as mybir
import concourse.tile as tile
from concourse.bass_utils import run_bass_kernel_spmd

dt = mybir.dt
AF = mybir.ActivationFunctionType
ALU = mybir.AluOpType
PM = mybir.MatmulPerfMode

P = 128
E = 1024
H = 16
D = 64
N_CORES = 8
EC = E // P   # 8 chunks of 128 along the embedding dim
G = EC // 2   # 4 DoubleRow chunk pairs
SBLK = 512    # s-block width for x loads / phase-C processing
XW = EC * SBLK  # flat x-block width (4096 fp8 bytes/partition)

SW = 32.0          # host weight pre-scale (std -> ~1)
SQ = 64.0          # qn scale, folded into invq
SK = 256.0         # W2 scale, folded into invk
INV_OUT = 1.0 / (SQ * SK)


# --------------------------------------------------------------------------
# BIR sync legalization: max one wait / one update per instruction.
# --------------------------------------------------------------------------
def _legalize_sync_json(bir_json: bytes) -> bytes:
    m = json.loads(bir_json)
    counter = [0]

    def fresh():
        counter[0] += 1
        return f"I-synclift-{counter[0]}"

    for f in m["functions"]:
        for blk in f["blocks"]:
            out = []
            for ins in blk["instructions"]:
                si = ins.get("sync_info")
                if not si:
                    out.append(ins)
                    continue
                waits = si.get("on_wait") or []
                updates = si.get("on_update") or []
                if len(waits) <= 1 and len(updates) <= 1:
                    out.append(ins)
                    continue
                eng = ins.get("engine")
                dbg = ins.get("debug")
                for w in waits[:-1]:
                    out.append(
                        {
                            "debug": dbg,
                            "engine": eng,
                            "ins": [],
                            "name": fresh(),
                            "opcode": "NoOp",
                            "outs": [],
                            "sync_info": {"on_update": [], "on_wait": [w]},
                        }
                    )
                si["on_wait"] = waits[-1:]
                post = [
                    {
                        "debug": dbg,
                        "engine": eng,
                        "ins": [],
                        "name": fresh(),
                        "opcode": "NoOp",
                        "outs": [],
                        "sync_info": {"on_update": [u], "on_wait": []},
                    }
                    for u in updates[1:]
                ]
                si["on_update"] = updates[:1]
                out.append(ins)
                out.extend(post)
            blk["instructions"] = out
    return json.dumps(m).encode()


def _patch_bass(nc):
    orig = nc.to_json_bytes

    def patched():
        return _legalize_sync_json(orig())

    nc.to_json_bytes = patched
    return nc


# --------------------------------------------------------------------------
# Ldweights dedup: the PE keeps the stationary operand loaded, so an
# InstLdweights that re-loads the exact AP the array already holds (with
# only matmuls in between, and no sync of its own) is dead.  Verified on
# hardware: the follow-on non-self-loading InstMatmult computes correctly.
# --------------------------------------------------------------------------
def _dedup_ldweights(nc):
    removed = 0
    for f in nc.m.functions:
        for blk in f.blocks:
            insns = list(blk.instructions)
            out = []
            last_w = None
            changed = False
            for ins in insns:
                tn = type(ins).__name__
                if str(getattr(ins, "engine", "")) == "EngineType.PE":
                    if tn == "InstLdweights":
                        si = ins.sync_info
                        has_sync = bool(
                            si and (list(si.on_wait) or list(si.on_update))
                        )
                        pap = ins.ins[0]
                        k = (
                            pap.memref, pap.offset, str(pap.ap),
                            str(pap.dtype), str(ins.perf_mode),
                            str(ins.is_transpose),
                        )
                        if k == last_w and not has_sync:
                            removed += 1
                            changed = True
                            continue
                        last_w = k
                    elif tn in ("InstMatmult", "InstRegisterMove"):
                        pass
                    else:
                        last_w = None
                out.append(ins)
            if changed:
                blk.instructions = out
    return removed


# --------------------------------------------------------------------------
# Kernel builder
# --------------------------------------------------------------------------
def build(S: int = 4096, with_bias: bool = True, cfg: dict | None = None):
    cfg = dict(cfg or {})
    ST = S // P      # number of 128-row s-tiles
    NBLK = S // SBLK  # number of 512-col s-blocks
    JB = SBLK // P   # s-tiles per block (4)

    nc = bass.Bass(trn_type="TRN2", target_bir_lowering=False, debug=False)

    f32 = dt.float32
    f32r = dt.float32r
    bf16 = dt.bfloat16
    f8 = dt.float8e4

    def xdram(name):
        return nc.dram_tensor(name, [P, NBLK * XW], f8, kind="ExternalInput").ap()

    def wdram(name):
        return nc.dram_tensor(name, [G * P, 2 * E], f8, kind="ExternalInput").ap()

    xq_hi_d, xq_lo_d = xdram("xq_hi"), xdram("xq_lo")
    xk_hi_d, xk_lo_d = xdram("xk_hi"), xdram("xk_lo")
    xv_hi_d, xv_lo_d = xdram("xv_hi"), xdram("xv_lo")
    wq_hi_d, wq_lo_d = wdram("wq_hi"), wdram("wq_lo")
    wk_hi_d, wk_lo_d = wdram("wk_hi"), wdram("wk_lo")
    wv_hi_d, wv_lo_d = wdram("wv_hi"), wdram("wv_lo")
    WoTd = nc.dram_tensor("WoT", [E, E], bf16, kind="ExternalInput").ap()
    bonesd = nc.dram_tensor("bones", [P, EC * H], f8, kind="ExternalInput").ap()
    bpickd = nc.dram_tensor("bpick", [H, EC * P], bf16, kind="ExternalInput").ap()
    bq = nc.dram_tensor("bq", [1, E], f32, kind="ExternalInput").ap()
    bk = nc.dram_tensor("bk", [1, E], f32, kind="ExternalInput").ap()
    bv = nc.dram_tensor("bv", [1, E], f32, kind="ExternalInput").ap()
    bo = nc.dram_tensor("bo", [1, E], f32, kind="ExternalInput").ap()
    out = nc.dram_tensor("out", [S, E], f32, kind="ExternalOutput").ap()

    def pair_view(t):
        return t[:].rearrange("p (i n) -> p i n", i=2)

    with tile.TileContext(nc) as tc:
        with (
            tc.tile_pool(name="consts", bufs=1) as consts,
            tc.tile_pool(name="small", bufs=1) as small,
            tc.tile_pool(name="drpool", bufs=1, space="DRAM") as drpool,
            tc.tile_pool(name="wts_o", bufs=1) as wts_o,
            tc.tile_pool(name="wts_q", bufs=1) as wts_q,
            tc.tile_pool(name="w2pool", bufs=1) as w2pool,
            tc.tile_pool(name="c_in", bufs=4) as c_in,
        ):
            # ---------------- constants ----------------
            zero128 = consts.tile([P, P], bf16, name="zero128")
            nc.gpsimd.memset(zero128[:], 0.0)
            ones_1x128 = None
            if with_bias:
                ones_st = consts.tile([1, P], f32, name="ones_st")
                nc.vector.memset(ones_st[:], 1.0)
                ones_1x128 = consts.tile([1, P], f32r, name="ones_1x128")
                nc.vector.tensor_copy(ones_1x128[:], ones_st[:])

            bones_sb = consts.tile([P, EC * H], f8, name="bones_sb")
            bpick_sb = consts.tile([H, EC * P], bf16, name="bpick_sb")

            def bones_pair(g):
                return bones_sb[:, 2 * g * H : (2 * g + 2) * H].rearrange(
                    "p (i h) -> p i h", i=2
                )

            # ---------------- biases ----------------
            rows_scope = tc.tile_pool(name="rows", bufs=1)
            rows_pool = rows_scope.__enter__()
            bk_row = bv_row = bo_bcast = bq_col = None
            if with_bias:
                with tc.tile_pool(name="brow_stage", bufs=2) as stage_pool:
                    def load_row_r(name, src, scale):
                        stage = stage_pool.tile([1, E], f32, name="brow_stage")
                        nc.sync.dma_start(stage[:], src)
                        row = rows_pool.tile([1, E], f32r, name=f"{name}_r")
                        if scale == 1.0:
                            nc.vector.tensor_copy(row[:], stage[:])
                        else:
                            nc.vector.tensor_scalar(
                                row[:], stage[:], scale, None, ALU.mult
                            )
                        return row

                    # phase-A biases enter the *scaled* psum, so pre-scale
                    bk_row = load_row_r("bk", bk, SW)
                    bv_row = load_row_r("bv", bv, SW)
                    bo_row = load_row_r("bo", bo, 1.0)

                bq_col = small.tile([P, EC], f32, name="bq_col")
                nc.sync.dma_start(bq_col[:], bq.rearrange("1 (t p) -> p t", p=P))

                with tc.tile_pool(name="bias_ps", bufs=2, space="PSUM") as bias_ps:
                    def bcast_row(row_r, name):
                        full = small.tile([P, E], f32, name=f"{name}_bcast")
                        for h in range(2):
                            pt = bias_ps.tile([P, 512], f32, name="bias_ps")
                            nc.tensor.matmul(
                                pt[:],
                                ones_1x128[:],
                                row_r[:, h * 512 : (h + 1) * 512],
                                start=True,
                                stop=True,
                            )
                            nc.vector.tensor_copy(
                                full[:, h * 512 : (h + 1) * 512], pt[:]
                            )
                        return full

                    bo_bcast = bcast_row(bo_row, "bo")

            def load_wpair(hid, lod, name, pool, q, interleave=False):
                his = [pool.tile([P, 2 * E], f8, name=f"{name}h_{g}")
                       for g in range(G)]
                los = [pool.tile([P, 2 * E], f8, name=f"{name}l_{g}")
                       for g in range(G)]
                if interleave:
                    # hi/lo per pair-group: the per-tile stationary-major
                    # matmul order needs W_lo[g] right after W_hi[g]
                    for g in range(G):
                        q.dma_start(his[g][:], hid[g * P : (g + 1) * P, :])
                        q.dma_start(los[g][:], lod[g * P : (g + 1) * P, :])
                else:
                    for g in range(G):
                        q.dma_start(his[g][:], hid[g * P : (g + 1) * P, :])
                    for g in range(G):
                        q.dma_start(los[g][:], lod[g * P : (g + 1) * P, :])
                return his, los

            bd_st = [
                small.tile([P, P], f32, name=f"bd_st_{pr}") for pr in range(8)
            ]
            odd_all = small.tile([D, 8 * D], f32, name="odd_all")

            def load_xq(blk_i):
                h_t = c_in.tile([P, XW], f8, name="xqh_blk")
                l_t = c_in.tile([P, XW], f8, name="xql_blk")
                nc.sync.dma_start(h_t[:], xq_hi_d[:, blk_i * XW : (blk_i + 1) * XW])
                nc.sync.dma_start(l_t[:], xq_lo_d[:, blk_i * XW : (blk_i + 1) * XW])
                return h_t, l_t

            def mm_dr(pj, lhsT, rhs, start, stop):
                nc.tensor.matmul(
                    pj, lhsT, rhs, start=start, stop=stop,
                    perf_mode=PM.DoubleRow,
                )

            # ================= PHASE A ====================================
            with (
                tc.tile_pool(name="wts_kv", bufs=1) as wts_kv,
                tc.tile_pool(name="a_in", bufs=2) as a_in,
                tc.tile_pool(name="a_act", bufs=2) as a_act,
                tc.tile_pool(name="a_kv", bufs=4) as a_kv,
                tc.tile_pool(name="a_pj_ps", bufs=6, space="PSUM") as a_pj_ps,
                tc.tile_pool(name="a_sc_ps", bufs=1, space="PSUM") as a_sc_ps,
            ):
                def load_xblk(hid, lod, blk_i, name):
                    h_t = a_in.tile([P, XW], f8, name=f"{name}h_blk")
                    l_t = a_in.tile([P, XW], f8, name=f"{name}l_blk")
                    nc.sync.dma_start(h_t[:], hid[:, blk_i * XW : (blk_i + 1) * XW])
                    nc.sync.dma_start(l_t[:], lod[:, blk_i * XW : (blk_i + 1) * XW])
                    return h_t, l_t

                # startup: xk block 0 on the SP queue, Wk/Wv hi+lo on the
                # ACT queue, Wo/Wq/bones/xq0 on the Pool queue.  The DMA
                # engines are shared, but per-queue dispatch lets the
                # scheduler start the PE as soon as xk_hi + Wk_hi land.
                xk_hi_b = a_in.tile([P, XW], f8, name="xkh_blk")
                nc.sync.dma_start(xk_hi_b[:], xk_hi_d[:, 0:XW])
                Wk_hi, Wk_lo = load_wpair(wk_hi_d, wk_lo_d, "Wk", wts_kv, nc.scalar)
                xk_lo_b = a_in.tile([P, XW], f8, name="xkl_blk")
                nc.sync.dma_start(xk_lo_b[:], xk_lo_d[:, 0:XW])
                xv_hi_b = a_in.tile([P, XW], f8, name="xvh_blk")
                nc.sync.dma_start(xv_hi_b[:], xv_hi_d[:, 0:XW])
                Wv_hi, Wv_lo = load_wpair(wv_hi_d, wv_lo_d, "Wv", wts_kv, nc.sync)
                xv_lo_b = a_in.tile([P, XW], f8, name="xvl_blk")
                nc.sync.dma_start(xv_lo_b[:], xv_lo_d[:, 0:XW])

                WoT = [wts_o.tile([P, E], bf16, name=f"WoT_{c}")
                       for c in range(EC)]
                Wq_hi = [wts_q.tile([P, 2 * E], f8, name=f"Wqh_{g}")
                         for g in range(G)]
                Wq_lo = [wts_q.tile([P, 2 * E], f8, name=f"Wql_{g}")
                         for g in range(G)]
                xq0_blks = (c_in.tile([P, XW], f8, name="xqh_blk"),
                            c_in.tile([P, XW], f8, name="xql_blk"))
                xq1_blks = (c_in.tile([P, XW], f8, name="xqh_blk"),
                            c_in.tile([P, XW], f8, name="xql_blk"))
                xq2_blks = (c_in.tile([P, XW], f8, name="xqh_blk"),
                            c_in.tile([P, XW], f8, name="xql_blk"))
                xq3_blks = (c_in.tile([P, XW], f8, name="xqh_blk"),
                            c_in.tile([P, XW], f8, name="xql_blk"))

                def staged_loads(it):
                    # one small DMA per iteration on the ACT queue so the
                    # weight transfers never block the ELU activations long
                    if it == 4:
                        nc.scalar.dma_start(bpick_sb[:], bpickd)
                    elif it == 5:
                        nc.scalar.dma_start(bones_sb[:], bonesd)
                    elif 6 <= it <= 13:
                        c = it - 6
                        nc.scalar.dma_start(
                            WoT[c][:], WoTd[c * P : (c + 1) * P, :]
                        )
                    elif 14 <= it <= 17:
                        g = it - 14
                        nc.scalar.dma_start(
                            Wq_hi[g][:], wq_hi_d[g * P : (g + 1) * P, :]
                        )
                    elif 18 <= it <= 21:
                        g = it - 18
                        nc.scalar.dma_start(
                            Wq_lo[g][:], wq_lo_d[g * P : (g + 1) * P, :]
                        )
                    elif 22 <= it <= 29:
                        xq_t = (xq0_blks, xq1_blks, xq2_blks, xq3_blks)[
                            (it - 22) // 2
                        ]
                        src_d = (xq_hi_d, xq_lo_d)[(it - 22) % 2]
                        blk = (it - 22) // 2
                        nc.scalar.dma_start(
                            xq_t[(it - 22) % 2][:],
                            src_d[:, blk * XW : (blk + 1) * XW],
                        )

                scores_ps = a_sc_ps.tile([P, H * D], f32, name="scores_ps")
                # 3 idempotent passes: keeps the PE p-state warm while the
                # first xk/Wk transfers land
                for _ in range(3):
                    for qtr in range(8):
                        nc.tensor.matmul(
                            scores_ps[:, qtr * P : (qtr + 1) * P],
                            zero128[:],
                            zero128[:],
                            start=True,
                            stop=True,
                            skip_group_check=True,
                        )

                def proj_tile(xh_b, xl_b, wh, wl, t, brow):
                    """Both output halves of one s-subtile, stationary-major
                    so the x-pair ldweights dedup 4x (hi) / 2x (lo)."""
                    xh = xh_b[:].rearrange("p (c s) -> p c s", c=EC)
                    xl = xl_b[:].rearrange("p (c s) -> p c s", c=EC)
                    pjs = [a_pj_ps.tile([P, 512], f32, name="pj")
                           for _ in range(2)]
                    for g in range(G):
                        st = xh[:, 2 * g : 2 * g + 2, t * P : (t + 1) * P]
                        for wb, h in ((wh, 0), (wh, 1), (wl, 0), (wl, 1)):
                            mm_dr(
                                pjs[h][:],
                                st,
                                pair_view(wb[g])[:, :, h * 512 : (h + 1) * 512],
                                start=(g == 0 and wb is wh),
                                stop=False,
                            )
                    for g in range(G):
                        st = xl[:, 2 * g : 2 * g + 2, t * P : (t + 1) * P]
                        for h in range(2):
                            mm_dr(
                                pjs[h][:],
                                st,
                                pair_view(wh[g])[:, :, h * 512 : (h + 1) * 512],
                                start=False,
                                stop=(brow is None and g == G - 1),
                            )
                    if brow is not None:
                        for h in range(2):
                            nc.tensor.matmul(
                                pjs[h][:], ones_1x128[:],
                                brow[:, h * 512 : (h + 1) * 512],
                                start=False, stop=True,
                            )
                    return pjs

                def k_elu_half(kv4, h, kp):
                    # bf16 intermediates: all-16-bit operands let the DVE
                    # run its 2x/4x perf modes
                    r_sb = a_act.tile([P, 512], bf16, name="kr_sb")
                    t_sb = a_act.tile([P, 512], bf16, name="kt_sb")
                    e_sb = a_act.tile([P, 512], bf16, name="ke_sb")
                    nc.scalar.activation(r_sb[:], kp[:], AF.Relu, scale=1.0 / SW)
                    # elu(x) = relu(x) + min(exp(x), 1) - 1
                    nc.scalar.activation(e_sb[:], kp[:], AF.Exp, scale=1.0 / SW)
                    nc.vector.tensor_scalar(
                        t_sb[:], e_sb[:], 1.0, -1.0, ALU.min, ALU.add
                    )
                    nc.vector.tensor_tensor(
                        kv4[:, 8 * h : 8 * (h + 1), D : 2 * D],
                        t_sb[:].rearrange("p (hh d) -> p hh d", d=D),
                        r_sb[:].rearrange("p (hh d) -> p hh d", d=D),
                        ALU.add,
                    )

                def do_kproj(it):
                    kv_sb = a_kv.tile([P, 2 * E], bf16, name="kv_sb")
                    kv4 = kv_sb[:].rearrange("p (hh two) -> p hh two", two=2 * D)
                    t = it % JB
                    kps = proj_tile(xk_hi_b, xk_lo_b, Wk_hi, Wk_lo, t, bk_row)
                    for h in range(2):
                        k_elu_half(kv4, h, kps[h])
                    return kv_sb, kv4

                def do_vproj(it, kv4):
                    t = it % JB
                    vps = proj_tile(xv_hi_b, xv_lo_b, Wv_hi, Wv_lo, t, bv_row)
                    for h in range(2):
                        nc.scalar.mul(
                            kv4[:, 8 * h : 8 * (h + 1), 0:D],
                            vps[h][:].rearrange("p (hh d) -> p hh d", d=D),
                            1.0 / SW,
                        )

                def do_scores(it, kv_sb):
                    for hh in range(H):
                        nc.tensor.matmul(
                            scores_ps[:, hh * D : (hh + 1) * D],
                            kv_sb[:, 2 * D * hh : 2 * D * (hh + 1)],
                            kv_sb[:, 2 * D * hh + D : 2 * D * (hh + 1)],
                            start=False,
                            stop=(it == ST - 1 and hh == H - 1),
                            skip_group_check=True,
                        )

                def warm_pair(ts_pair, xh_b, xl_b, wh, wl, brow):
                    """pass-major projection for two block-0 subtiles so
                    the PE starts on hi*hi as soon as W_hi lands."""
                    xh = xh_b[:].rearrange("p (c s) -> p c s", c=EC)
                    xl = xl_b[:].rearrange("p (c s) -> p c s", c=EC)
                    pjs = {t: [a_pj_ps.tile([P, 512], f32, name="pj")
                               for _ in range(2)] for t in ts_pair}
                    # hi*hi first (needs only W_hi), then hi*lo, then lo*hi
                    for pi, (xa, wb) in enumerate(
                        ((xh, wh), (xh, wl), (xl, wh))
                    ):
                        for t in ts_pair:
                            for g in range(G):
                                st = xa[:, 2 * g : 2 * g + 2,
                                        t * P : (t + 1) * P]
                                for h in range(2):
                                    mm_dr(
                                        pjs[t][h][:],
                                        st,
                                        pair_view(wb[g])[
                                            :, :, h * 512 : (h + 1) * 512
                                        ],
                                        start=(pi == 0 and g == 0),
                                        stop=(brow is None and pi == 2
                                              and g == G - 1),
                                    )
                    for t in ts_pair:
                        if brow is not None:
                            for h in range(2):
                                nc.tensor.matmul(
                                    pjs[t][h][:], ones_1x128[:],
                                    brow[:, h * 512 : (h + 1) * 512],
                                    start=False, stop=True,
                                )
                    return pjs

                kv_b0 = {}
                for ts_pair in ((0, 1), (2, 3)):
                    kps = warm_pair(ts_pair, xk_hi_b, xk_lo_b,
                                    Wk_hi, Wk_lo, bk_row)
                    for t in ts_pair:
                        kv_sb = a_kv.tile([P, 2 * E], bf16, name="kv_sb")
                        kv4 = kv_sb[:].rearrange(
                            "p (hh two) -> p hh two", two=2 * D
                        )
                        kv_b0[t] = (kv_sb, kv4)
                        for h in range(2):
                            k_elu_half(kv4, h, kps[t][h])
                for ts_pair in ((0, 1), (2, 3)):
                    vps = warm_pair(ts_pair, xv_hi_b, xv_lo_b,
                                    Wv_hi, Wv_lo, bv_row)
                    for t in ts_pair:
                        kv4 = kv_b0[t][1]
                        for h in range(2):
                            nc.scalar.mul(
                                kv4[:, 8 * h : 8 * (h + 1), 0:D],
                                vps[t][h][:].rearrange(
                                    "p (hh d) -> p hh d", d=D
                                ),
                                1.0 / SW,
                            )
                for t in range(JB):
                    do_scores(t, kv_b0[t][0])

                xk_nxt = xv_nxt = None
                xk_nxt = load_xblk(xk_hi_d, xk_lo_d, 1, "xk")
                xv_nxt = load_xblk(xv_hi_d, xv_lo_d, 1, "xv")
                for it in range(JB, ST):
                    blk_i, t = divmod(it, JB)
                    if t == 0:
                        (xk_hi_b, xk_lo_b) = xk_nxt
                        (xv_hi_b, xv_lo_b) = xv_nxt
                    if t == 0 and blk_i + 1 < NBLK:
                        xk_nxt = load_xblk(xk_hi_d, xk_lo_d, blk_i + 1, "xk")
                        xv_nxt = load_xblk(xv_hi_d, xv_lo_d, blk_i + 1, "xv")

                    staged_loads(it)
                    kv_sb, kv4 = do_kproj(it)
                    do_vproj(it, kv4)
                    do_scores(it, kv_sb)

                for s_t in bd_st:
                    nc.vector.memset(s_t[:], 0.0)

                # -- extract scoresT + ksumsq while phase-A psum still alive
                # Gram rows (64:128) hold k^T k per head; diagonal = ksumsq
                gram_sb = small.tile([D, H * D], f32, name="gram_sb")
                nc.vector.tensor_copy(gram_sb[:], scores_ps[D:P, :])
                gram_dram = drpool.tile([1, D * H * D], f32, name="gram_dram")
                nc.sync.dma_start(
                    gram_dram[:].rearrange("1 (d c) -> d c", d=D), gram_sb[:]
                )
                # diag idx for (hh, d) = d*(H*D) + hh*D + d = d*(H*D+1) + D*hh
                kcol = small.tile([P, EC], f32, name="kcol")
                gd = gram_dram[:].tensor
                for h2 in range(2):
                    src_ap = bass.AP(
                        gd, h2 * D, [[H * D + 1, D], [2 * D, EC]]
                    )
                    nc.sync.dma_start(kcol[h2 * D : (h2 + 1) * D, :], src_ap)
                # knorm' = sqrt(kcol * S) / SK  (scaled so W2 ~ std 1)
                knorm = small.tile([P, EC], f32, name="knorm")
                nc.scalar.activation(
                    knorm[:], kcol[:], AF.Sqrt, scale=float(S) / (SK * SK)
                )
                invk = small.tile([P, EC], f32, name="invk")
                nc.vector.reciprocal(invk[:], knorm[:])

                # stage all odd-head blocks at once: one strided DMA
                # shifts them to partitions 64:128 of the bd tiles
                for pr in range(8):
                    nc.scalar.copy(
                        bd_st[pr][0:D, 0:D],
                        scores_ps[0:D, 2 * pr * D : (2 * pr + 1) * D],
                    )
                    nc.scalar.copy(
                        odd_all[:, pr * D : (pr + 1) * D],
                        scores_ps[0:D, (2 * pr + 1) * D : (2 * pr + 2) * D],
                    )
                bd_dram = drpool.tile([1, D * 8 * D], f32, name="bd_dram")
                nc.sync.dma_start(
                    bd_dram[:].rearrange("1 (d c) -> d c", d=D), odd_all[:]
                )
                bdd = bd_dram[:].tensor
                for pr in range(8):
                    nc.sync.dma_start(
                        bd_st[pr][D:P, D:P],
                        bass.AP(bdd, pr * D, [[8 * D, D], [1, D]]),
                    )
                bd = []
                for pr in range(8):
                    bd_t = small.tile([P, P], bf16, name=f"bd_{pr}")
                    nc.gpsimd.tensor_copy(bd_t[:], bd_st[pr][:])
                    bd.append(bd_t)

            rows_scope.__exit__(None, None, None)

            # ============ PHASE B + C: software-pipelined q pass ==========
            W2h = [w2pool.tile([P, 2 * E], f8, name=f"W2h_{g}") for g in range(G)]
            W2l = [w2pool.tile([P, 2 * E], f8, name=f"W2l_{g}") for g in range(G)]
            with (
                tc.tile_pool(name="c_qt", bufs=2) as c_qt,
                tc.tile_pool(name="c_qn", bufs=3) as c_qn,
                tc.tile_pool(name="c_q2", bufs=2) as c_q2,
                tc.tile_pool(name="c_tmp", bufs=2) as c_tmp,
                tc.tile_pool(name="c_out", bufs=2) as c_out,
                tc.tile_pool(name="c_pj_ps", bufs=3, space="PSUM") as c_pj_ps,
                tc.tile_pool(name="c_fin_ps", bufs=4, space="PSUM") as c_fin_ps,
                tc.tile_pool(name="c_ss_ps", bufs=1, space="PSUM") as c_ss_ps,
                tc.tile_pool(name="c_qb", bufs=2) as c_qb,
                tc.tile_pool(name="c_dr", bufs=2, space="DRAM") as c_dr,
            ):
                xq_pre = {1: xq1_blks, 2: xq2_blks, 3: xq3_blks}

                def projA(blk_i, xq_blks=None):
                    """q projection (qT layout) + ELU + row sum-of-squares.
                    W-stationary-major: Wq_hi slices dedup their ldweights
                    across the x_hi and x_lo passes."""
                    if xq_blks is None:
                        xq_blks = xq_pre.pop(blk_i, None) or load_xq(blk_i)
                    for pre_i in (blk_i + 1, blk_i + 2):
                        if pre_i < NBLK and pre_i not in xq_pre:
                            xq_pre[pre_i] = load_xq(pre_i)
                    xh_b, xl_b = xq_blks
                    xh = xh_b[:].rearrange("p (c s) -> p c s", c=EC)
                    xl = xl_b[:].rearrange("p (c s) -> p c s", c=EC)
                    qt_tiles = []
                    q2p = [c_q2.tile([P, 2 * SBLK], f8, name=f"q2p_{g}")
                           for g in range(G)]
                    for ot in range(EC):
                        pj = c_pj_ps.tile([P, SBLK], f32, name="q_pj")
                        for g in range(G):
                            st = pair_view(Wq_hi[g])[:, :, ot * P : (ot + 1) * P]
                            mm_dr(pj[:], st, xh[:, 2 * g : 2 * g + 2, :],
                                  start=(g == 0), stop=False)
                            mm_dr(pj[:], st, xl[:, 2 * g : 2 * g + 2, :],
                                  start=False, stop=False)
                        for g in range(G):
                            mm_dr(
                                pj[:],
                                pair_view(Wq_lo[g])[:, :, ot * P : (ot + 1) * P],
                                xh[:, 2 * g : 2 * g + 2, :],
                                start=False, stop=(g == G - 1),
                            )
                        r_sb = c_tmp.tile([P, SBLK], bf16, name="qr_sb")
                        t_sb = c_tmp.tile([P, SBLK], bf16, name="qt_sb")
                        e_sb = c_tmp.tile([P, SBLK], bf16, name="qe_sb")
                        qt_ = c_qt.tile([P, SBLK], bf16, name=f"qt_{ot}")
                        qbias = bq_col[:, ot : ot + 1] if with_bias else 0.0
                        nc.scalar.activation(
                            r_sb[:], pj[:], AF.Relu, bias=qbias, scale=1.0 / SW
                        )
                        # elu(x) = relu(x) + min(exp(x), 1) - 1
                        nc.scalar.activation(
                            e_sb[:], pj[:], AF.Exp, bias=qbias, scale=1.0 / SW
                        )
                        nc.vector.tensor_scalar(
                            t_sb[:], e_sb[:], 1.0, -1.0, ALU.min, ALU.add
                        )
                        nc.vector.tensor_tensor(
                            qt_[:], t_sb[:], r_sb[:], ALU.add
                        )
                        qt_tiles.append(qt_)
                        g, i = ot // 2, ot % 2
                        with nc.allow_low_precision(
                            reason="q^2 rounds to fp8; the 64-term qss sum "
                                   "averages the noise to ~0.3%"
                        ):
                            (nc.vector if ot % 2 else nc.gpsimd).tensor_tensor(
                                pair_view(q2p[g])[:, i, :], qt_[:], qt_[:],
                                ALU.mult,
                            )
                    return qt_tiles, q2p

                def finish_ss(state):
                    """qss matmuls + sqrt + reciprocal + invq spill: the
                    head of the qn chain, emitted late so the qss matmuls
                    never stall the PE on the Pool-engine q2 tiles."""
                    qt_tiles, q2p = state
                    qss_ps = c_ss_ps.tile([H, SBLK], f32, name="qss_ps")
                    for g in range(G):
                        mm_dr(
                            qss_ps[:], bones_pair(g), pair_view(q2p[g]),
                            start=(g == 0), stop=(g == G - 1),
                        )
                    # qss_sb = sqrt(D * qss) / SQ  (so invq = SQ/||q||sqrt(D))
                    qss_sb = c_tmp.tile([H, SBLK], f32, name="qss_sb")
                    nc.scalar.activation(
                        qss_sb[:], qss_ps[:], AF.Sqrt, scale=float(D) / (SQ * SQ)
                    )
                    invq_b = c_tmp.tile([H, SBLK], bf16, name="invq_b")
                    with nc.allow_low_precision(
                        reason="invq rounds to bf16; the fp8 qn quantization "
                               "dominates"
                    ):
                        nc.vector.reciprocal(invq_b[:], qss_sb[:])
                    invq_dr = c_dr.tile([1, H * SBLK], bf16, name="invq_dr")
                    nc.gpsimd.dma_start(
                        invq_dr[:].rearrange("1 (h s) -> h s", h=H), invq_b[:]
                    )
                    return qt_tiles, invq_dr, invq_b

                def normB(state):
                    """invq broadcast (fused stride-0 DMAs on the Pool
                    queue) + scale to fp8 hi/lo qn pair tiles."""
                    qt_tiles, invq_dr, _ = state
                    dr = invq_dr[:].tensor
                    qb_all = c_qb.tile([P, EC * SBLK], bf16, name="qb_sb")
                    for hf in range(2):
                        nc.gpsimd.dma_start(
                            qb_all[hf * D : (hf + 1) * D, :].rearrange(
                                "p (c s) -> p c s", c=EC
                            ),
                            bass.AP(
                                dr, hf * SBLK,
                                [[0, D], [2 * SBLK, EC], [1, SBLK]],
                            ),
                        )
                    qnh = [c_qn.tile([P, 2 * SBLK], f8, name=f"qnh_{g}")
                           for g in range(G)]
                    qnl = [c_qn.tile([P, 2 * SBLK], f8, name=f"qnl_{g}")
                           for g in range(G)]
                    qnfs = []
                    with nc.allow_low_precision(
                        reason="qn in bf16 then fp8 hi/lo; residual floor "
                               "is bf16-level"
                    ):
                        # hi tiles first (consume reads them before the lo
                        # tiles), lo subtractions afterwards
                        for ot in range(EC):
                            g, i = ot // 2, ot % 2
                            qb = qb_all[:, ot * SBLK : (ot + 1) * SBLK]
                            qnf = c_tmp.tile([P, SBLK], bf16, name=f"qnf_{ot % 4}")
                            nc.vector.tensor_tensor(
                                qnf[:], qt_tiles[ot][:], qb, ALU.mult
                            )
                            nc.scalar.copy(pair_view(qnh[g])[:, i, :], qnf[:])
                            qnfs.append(qnf)
                        for ot in range(EC):
                            g, i = ot // 2, ot % 2
                            nc.vector.tensor_tensor(
                                pair_view(qnl[g])[:, i, :], qnfs[ot][:],
                                pair_view(qnh[g])[:, i, :], ALU.subtract,
                            )
                    return qnh, qnl

                def normB_pe(state):
                    """tail-block qn: invq broadcast via PE bpick matmuls
                    so the chain has no DMA roundtrip latency."""
                    qt_tiles, _, invq_b = state
                    qnh = [c_qn.tile([P, 2 * SBLK], f8, name=f"qnh_{g}")
                           for g in range(G)]
                    qnl = [c_qn.tile([P, 2 * SBLK], f8, name=f"qnl_{g}")
                           for g in range(G)]
                    qnfs = []
                    with nc.allow_low_precision(
                        reason="qn in bf16 then fp8 hi/lo; residual floor "
                               "is bf16-level"
                    ):
                        for ot in range(EC):
                            g, i = ot // 2, ot % 2
                            qb_ps = c_pj_ps.tile([P, SBLK], f32, name="q_pj")
                            nc.tensor.matmul(
                                qb_ps[:],
                                bpick_sb[:, ot * P : (ot + 1) * P],
                                invq_b[:],
                                start=True, stop=True,
                            )
                            qnf = c_tmp.tile([P, SBLK], bf16,
                                             name=f"qnf_{ot % 4}")
                            nc.vector.tensor_tensor(
                                qnf[:], qt_tiles[ot][:], qb_ps[:], ALU.mult
                            )
                            nc.scalar.copy(pair_view(qnh[g])[:, i, :], qnf[:])
                            qnfs.append(qnf)
                        for ot in range(EC):
                            g, i = ot // 2, ot % 2
                            nc.vector.tensor_tensor(
                                pair_view(qnl[g])[:, i, :], qnfs[ot][:],
                                pair_view(qnh[g])[:, i, :], ALU.subtract,
                            )
                    return qnh, qnl

                def consume(blk_i, qn_state):
                    """out = qnT.T @ W2 (+ bo) in natural layout.
                    qn-stationary-major: qnh slices dedup 4x, qnl 2x."""
                    qnh, qnl = qn_state
                    s0 = blk_i * SBLK
                    fine_tail = blk_i == NBLK - 1

                    for j in range(JB):
                        fins = [c_fin_ps.tile([P, 512], f32, name="fin_ps")
                                for _ in range(2)]
                        for g in range(G):
                            st = pair_view(qnh[g])[:, :, j * P : (j + 1) * P]
                            for wb, h in (
                                (W2h, 0), (W2h, 1), (W2l, 0), (W2l, 1)
                            ):
                                mm_dr(
                                    fins[h][:],
                                    st,
                                    pair_view(wb[g])[:, :, h * 512 : (h + 1) * 512],
                                    start=(g == 0 and wb is W2h),
                                    stop=False,
                                )
                        for g in range(G):
                            st = pair_view(qnl[g])[:, :, j * P : (j + 1) * P]
                            for h in range(2):
                                mm_dr(
                                    fins[h][:],
                                    st,
                                    pair_view(W2h[g])[:, :, h * 512 : (h + 1) * 512],
                                    start=False, stop=(g == G - 1),
                                )
                        o_sb = c_out.tile([P, E], f32, name="o_sb")
                        for h in range(2):
                            sl = slice(h * 512, (h + 1) * 512)
                            if with_bias:
                                nc.vector.scalar_tensor_tensor(
                                    o_sb[:, sl], fins[h][:], INV_OUT,
                                    bo_bcast[:, sl], ALU.mult, ALU.add,
                                )
                            elif h == 0:
                                nc.vector.tensor_scalar(
                                    o_sb[:, sl], fins[h][:], INV_OUT, None,
                                    ALU.mult,
                                )
                            else:
                                nc.scalar.mul(o_sb[:, sl], fins[h][:], INV_OUT)
                            if fine_tail and j == JB - 1:
                                nc.sync.dma_start(
                                    out[s0 + j * P : s0 + (j + 1) * P, sl],
                                    o_sb[:, sl],
                                )
                        if not (fine_tail and j == JB - 1):
                            nc.sync.dma_start(
                                out[s0 + j * P : s0 + (j + 1) * P, :], o_sb[:]
                            )

                # pipeline: projA(0) | B | normB(0) | projA(1) consume(0)
                # normB(1) | projA(2) consume(1) normB(2) | ... | consume(7)
                st0 = projA(0, xq0_blks)

                # ---- PHASE B: W2 hi/lo (psum borrowed from c_fin pool) ----
                def phaseB(cs):
                    for c in cs:
                        g, i = c // 2, c % 2
                        w2ps = [c_fin_ps.tile([P, 512], f32, name="fin_ps")
                                for _ in range(2)]
                        for h in range(2):
                            nc.tensor.matmul(
                                w2ps[h][:],
                                bd[c][:],
                                WoT[c][:, h * 512 : (h + 1) * 512],
                                start=True,
                                stop=True,
                            )
                        for h in range(2):
                            hi_sl = pair_view(W2h[g])[
                                :, i, h * 512 : (h + 1) * 512
                            ]
                            lo_sl = pair_view(W2l[g])[
                                :, i, h * 512 : (h + 1) * 512
                            ]
                            # hi on ACT, lo on DVE: splits the W2 build
                            # across queues at the transition (Pool cannot
                            # read PSUM)
                            nc.scalar.mul(
                                hi_sl, w2ps[h][:], invk[:, c : c + 1]
                            )
                            nc.vector.scalar_tensor_tensor(
                                lo_sl, w2ps[h][:], invk[:, c : c + 1], hi_sl,
                                ALU.mult, ALU.subtract,
                            )

                qn_done = {0: normB(finish_ss(st0))}
                phaseB(range(EC))
                st1 = projA(1)
                qn_done[1] = normB(finish_ss(st1))
                # consume lags projA by TWO blocks so the tail qn chain
                # drains under ~21us of PE work; the PE queue is in-order,
                # so the tail chain's PE pieces (qss, bpick-qb) interleave
                # BETWEEN the final consume calls to avoid head-of-line
                # stalls on the ELU->sqrt->recip chain
                for blk_i in range(2, NBLK - 1):
                    st_cur = projA(blk_i)
                    consume(blk_i - 2, qn_done.pop(blk_i - 2))
                    qn_done[blk_i] = normB(finish_ss(st_cur))
                st7 = projA(NBLK - 1)
                consume(NBLK - 3, qn_done.pop(NBLK - 3))
                fin7 = finish_ss(st7)
                consume(NBLK - 2, qn_done.pop(NBLK - 2))
                qn_done[NBLK - 1] = normB_pe(fin7)
                consume(NBLK - 1, qn_done.pop(NBLK - 1))

    _dedup_ldweights(nc)
    _patch_bass(nc)
    return nc


# --------------------------------------------------------------------------
# Host wrapper
# --------------------------------------------------------------------------
_NC_CACHE = {}


def _get_nc(S, with_bias=True):
    key = (S, with_bias)
    if key not in _NC_CACHE:
        _NC_CACHE[key] = build(S, with_bias)
    return _NC_CACHE[key]


def _fp8_hilo(x):
    import ml_dtypes

    e4 = ml_dtypes.float8_e4m3
    hi = x.astype(e4)
    lo = (x - hi.astype(np.float32)).astype(e4)
    return hi, lo


def _tile_x(x8):
    """[S, E] -> pre-tiled [128, S*E/128] with layout [p, blk, c, s]."""
    S = x8.shape[0]
    nblk = S // SBLK
    return np.ascontiguousarray(
        x8.reshape(nblk, SBLK, EC, P).transpose(3, 0, 2, 1).reshape(P, -1)
    )


def _pair_w(w8):
    """[E, E] fp8 (row-major W.T) -> DoubleRow pair layout [G*128, 2*E]."""
    return np.ascontiguousarray(
        w8.reshape(G, 2, P, E).transpose(0, 2, 1, 3).reshape(G * P, 2 * E)
    )


def make_in_maps(query, key, value, Wq, bq, Wk, bk, Wv, bv, Wo, bo):
    import ml_dtypes

    bf = ml_dtypes.bfloat16
    e4 = ml_dtypes.float8_e4m3
    B = np.asarray(query).shape[0]
    bones = np.zeros((P, EC * H), np.float32)
    bpick = np.zeros((H, EC * P), np.float32)
    for c in range(EC):
        bones[0:D, c * H + 2 * c] = 1.0
        bones[D:P, c * H + 2 * c + 1] = 1.0
        bpick[2 * c, c * P : c * P + D] = 1.0
        bpick[2 * c + 1, c * P + D : (c + 1) * P] = 1.0

    def wpair(W):
        Ws = np.asarray(W, np.float32).T * SW
        hi, lo = _fp8_hilo(Ws)
        return _pair_w(hi), _pair_w(lo)

    wq_hi, wq_lo = wpair(Wq)
    wk_hi, wk_lo = wpair(Wk)
    wv_hi, wv_lo = wpair(Wv)
    shared = {
        "bones": np.ascontiguousarray(bones.astype(e4)),
        "bpick": np.ascontiguousarray(bpick.astype(bf)),
        "wq_hi": wq_hi, "wq_lo": wq_lo,
        "wk_hi": wk_hi, "wk_lo": wk_lo,
        "wv_hi": wv_hi, "wv_lo": wv_lo,
        "WoT": np.ascontiguousarray(np.asarray(Wo, np.float32).T.astype(bf)),
        "bq": np.ascontiguousarray(np.asarray(bq, np.float32).reshape(1, E)),
        "bk": np.ascontiguousarray(np.asarray(bk, np.float32).reshape(1, E)),
        "bv": np.ascontiguousarray(np.asarray(bv, np.float32).reshape(1, E)),
        "bo": np.ascontiguousarray(np.asarray(bo, np.float32).reshape(1, E)),
    }

    def xmaps(x):
        hi, lo = _fp8_hilo(np.asarray(x, np.float32))
        return _tile_x(hi), _tile_x(lo)

    maps = []
    for c in range(B):
        xq_hi, xq_lo = xmaps(query[c])
        xk_hi, xk_lo = xmaps(key[c])
        xv_hi, xv_lo = xmaps(value[c])
        maps.append(
            {
                "xq_hi": xq_hi, "xq_lo": xq_lo,
                "xk_hi": xk_hi, "xk_lo": xk_lo,
                "xv_hi": xv_hi, "xv_lo": xv_lo,
                **shared,
            }
        )
    return maps


def kernel(query, key, value, Wq, bq, Wk, bk, Wv, bv, Wo, bo):
    query = np.asarray(query, np.float32)
    B, S, E_ = query.shape
    assert E_ == E and B == N_CORES
    in_maps = make_in_maps(query, key, value, Wq, bq, Wk, bk, Wv, bv, Wo, bo)
    with_bias = any(
        np.any(np.asarray(b)) for b in (bq, bk, bv, bo)
    )
    nc = _get_nc(S, with_bias)
    res = run_bass_kernel_spmd(nc, in_maps, core_ids=list(range(N_CORES)))
    return np.stack([res.results[c]["out"] for c in range(B)])
